# revision 9
# baseline (speedup 1.0000x reference)
"""DLRM (nn_DLRMNet) Trainium2 Bass kernel — 8-core SPMD, bf16 main loop.

Strategy:
  * Bottom MLP + BN replicated on every core, feature-major layout (BN reduces
    over the free/batch axis), fp32.
  * 26 embedding tables sharded across cores (contiguous blocks, 4/4/3/3/3/3/3/3);
    each core gathers its tables (bf16) for the full batch via indirect DMA,
    results AllGathered feature-major in bf16.
  * Pairwise interaction over cyclic diagonals: diag d of the f f^T outer
    product is f[i] * f[(i+d) % 432]; diags 0..216 cover the upper triangle
    exactly once (d=216 twice -> its W0 columns are halved). The 217 diagonals
    are sharded across cores (28/27x7, padded to 28). The per-core shift enters
    through *data* (indirect-DMA row gathers from a feature-major bf16 staging
    buffer), so the compiled program is identical on every core (SPMD).
  * The huge top W0 [512, 93544] is column-permuted to diagonal-pair order on
    the host, row-sharded [12288, 512] per core, cast to bf16, and loaded
    WHOLE into SBUF once (12.6 MB) overlapped with the bottom MLP.
  * Main loop is batch-tile OUTER / k-group inner: per batch tile the y0
    partial accumulates across all 97 matmuls (dense W0 part + 96 interaction
    chunks) in a single PSUM bank — no SBUF accumulation adds. Interaction
    chunks are built batch-major on DVE in bf16 and PE-transposed to
    pair-major (bf16 transposes run at 1 cycle/row vs 1.5 for fp32r).
  * The y0 AllReduce is split into two batch halves: the first half's
    collective overlaps the second half's compute.
  * Top MLP + BN replicated after the fp32 AllReduce.

Self-contained: only numpy / jax / concourse imports, shapes hardcoded.
"""
import numpy as np
from contextlib import ExitStack

import concourse.bass as bass
import concourse.mybir as mybir
import concourse.tile as tile

F32 = mybir.dt.float32
F32R = mybir.dt.float32r
BF16 = mybir.dt.bfloat16
I32 = mybir.dt.int32
AX = mybir.AxisListType
ALU = mybir.AluOpType
ACTF = mybir.ActivationFunctionType

# ---------------- problem constants ----------------
B, ND, NT, V, D = 1024, 13, 26, 100000, 16
F = 432                      # 16 + 26*16 interaction feature dim
NCORE = 8
EPS = 1e-5
NBT = 8                      # batch tiles of 128
P = 128

# table sharding: contiguous blocks
TCNT = [4, 4, 3, 3, 3, 3, 3, 3]
T0 = [0, 4, 8, 11, 14, 17, 20, 23]
SLOTS = 4                    # padded table slots per core
ESTACK = SLOTS * V           # rows in per-core padded table stack

# diagonal sharding: 217 cyclic diagonals, core0 28 real, others 27
DPC = 28                     # structural diags per core (uniform program)
D_BASE = [0, 28, 55, 82, 109, 136, 163, 190]
N_REAL = [28, 27, 27, 27, 27, 27, 27, 27]
SHARD_K = 12288              # 28*432 = 12096 padded to 96 chunks of 128
NKGP = 12                    # k-groups of 1024 pairs (8 chunks)
FDT_ROWS = 656               # feature-major staging: 448 + 200 dup + pad

_CACHE = {}
NPBF16 = mybir.dt.np(BF16)


# =====================================================================
# host-side input prep
# =====================================================================
def _triu_index(a, b):
    return a * F - a * (a - 1) // 2 + (b - a)


def _prep_core(c, inp):
    w0 = np.asarray(inp["w_t0"], np.float32)          # [512, 93544]
    w0i = w0[:, 16:]                                   # [512, 93528]

    p = np.arange(SHARD_K)
    m = p // F
    i = p % F
    d = D_BASE[c] + m
    valid = m < N_REAL[c]
    j = (i + d) % F
    a = np.minimum(i, j)
    b = np.maximum(i, j)
    pidx = np.where(valid, _triu_index(a, b), 0)
    w0t = w0i[:, pidx].T.astype(np.float32).copy()     # [12288, 512]
    w0t[~valid] = 0.0
    w0t[valid & (d == 216)] *= 0.5

    sparse = np.asarray(inp["sparse_features"])
    gidx_full = np.zeros((B, SLOTS), np.int32)
    for s in range(SLOTS):
        if s < TCNT[c]:
            gidx_full[:, s] = s * V + sparse[:, T0[c] + s].astype(np.int64)
    # [128, 32]: (partition, bt*4+slot)
    gidx = np.zeros((P, NBT * SLOTS), np.int32)
    for bt in range(NBT):
        gidx[:, bt * SLOTS:(bt + 1) * SLOTS] = gidx_full[bt * P:(bt + 1) * P]

    emb = np.asarray(inp["emb"], np.float32)           # [26, V, 16]
    estack = np.zeros((ESTACK, D), NPBF16)
    nc_t = TCNT[c]
    estack[: nc_t * V] = emb[T0[c]: T0[c] + nc_t].reshape(nc_t * V, D).astype(NPBF16)

    qp = np.arange(P)
    sidx = np.zeros((P, 4), np.int32)
    fidx = np.zeros((P, 4), np.int32)
    for q in range(4):
        v = D_BASE[c] + 128 * q + qp
        sidx[:, q] = np.where(v <= D_BASE[c] + 458, v, 0)
        v2 = 128 * q + qp
        fidx[:, q] = np.where(v2 < F, v2, 0)

    m_ = {
        "w0t": w0t.astype(NPBF16),
        "w0d": (w0[:, :16].T.copy().astype(NPBF16) if c == 0
                else np.zeros((16, 512), NPBF16)),
        "estack": estack,
        "gidx": gidx,
        "sidx": sidx,
        "fidx": fidx,
        "xT": np.asarray(inp["dense_features"], np.float32).T.copy(),
        "wb0t": np.asarray(inp["w_b0"], np.float32).T.copy(),
        "wb1t": np.asarray(inp["w_b1"], np.float32).T.astype(NPBF16).copy(),
        "wb2t": np.asarray(inp["w_b2"], np.float32).T.astype(NPBF16).copy(),
        "wt1t": np.asarray(inp["w_t1"], np.float32).T.astype(NPBF16).copy(),
        "wt2t": np.asarray(inp["w_t2"], np.float32).T.astype(NPBF16).copy(),
        "gb0": np.asarray(inp["g_b0"], np.float32).reshape(-1, 1),
        "beb0": np.asarray(inp["be_b0"], np.float32).reshape(-1, 1),
        "gb1": np.asarray(inp["g_b1"], np.float32).reshape(-1, 1),
        "beb1": np.asarray(inp["be_b1"], np.float32).reshape(-1, 1),
        "gb2": np.asarray(inp["g_b2"], np.float32).reshape(-1, 1),
        "beb2": np.asarray(inp["be_b2"], np.float32).reshape(-1, 1),
        "gt0": np.asarray(inp["g_t0"], np.float32).reshape(-1, 1),
        "bet0": np.asarray(inp["be_t0"], np.float32).reshape(-1, 1),
        "gt1": np.asarray(inp["g_t1"], np.float32).reshape(-1, 1),
        "bet1": np.asarray(inp["be_t1"], np.float32).reshape(-1, 1),
        "bt2": np.asarray(inp["b_t2"], np.float32).reshape(1, 1),
        "ident": np.eye(P, dtype=np.float32),
    }
    return m_


# =====================================================================
# post-Tile pass: split multi-waits (walrus wait-slot limits)
# =====================================================================
_WFX = [0]


def _split_waits(nc):
    n = 0
    for bbw in nc.bb_map.values():
        bb = bbw.bb
        new = []
        changed = False
        for inst in bb.instructions:
            si = inst.sync_info
            if si is not None and len(si.on_wait) > 1:
                waits = list(si.on_wait)
                for w in waits[:-1]:
                    _WFX[0] += 1
                    nop = mybir.InstNoOp(name=f"I-wfx-{_WFX[0]}", ins=[], outs=[])
                    nop.engine = inst.engine
                    nop.sync_info = mybir.SyncInfo(on_wait=[w], on_update=[])
                    new.append(nop)
                si.on_wait = waits[-1:]
                n += 1
                changed = True
            new.append(inst)
        if changed:
            bb.instructions = new
    return n


# =====================================================================
# bass program
# =====================================================================
def _bn_relu(nc, pool, src_f32, width, g_ap, be_ap, out_t):
    """src_f32: SBUF [p, width] fp32 -> out_t = relu(bn(src)).

    BN stats over the free axis (full batch must be in `width`).
    """
    s_sum = pool.tile([P, 1], F32, tag="bn_sum", name="bn_sum")
    s_sq = pool.tile([P, 1], F32, tag="bn_sq", name="bn_sq")
    pp = src_f32.shape[0]
    nc.vector.tensor_reduce(out=s_sum[:pp], in_=src_f32, axis=AX.X, op=ALU.add)
    sq_scratch = pool.tile([P, src_f32.shape[-1]], F32, tag="bn_sqscr", name="bn_sqscr", bufs=1)
    nc.scalar.activation(out=sq_scratch[:pp, : src_f32.shape[-1]], in_=src_f32,
                         func=ACTF.Square, accum_out=s_sq[:pp])
    mean = pool.tile([P, 1], F32, tag="bn_mean", name="bn_mean")
    var = pool.tile([P, 1], F32, tag="bn_var", name="bn_var")
    sd = pool.tile([P, 1], F32, tag="bn_sd", name="bn_sd")
    r = pool.tile([P, 1], F32, tag="bn_r", name="bn_r")
    s1 = pool.tile([P, 1], F32, tag="bn_s1", name="bn_s1")
    s2 = pool.tile([P, 1], F32, tag="bn_s2", name="bn_s2")
    t = pool.tile([P, 1], F32, tag="bn_t", name="bn_t")
    inv_n = 1.0 / width
    nc.vector.tensor_scalar_mul(out=mean[:pp], in0=s_sum[:pp], scalar1=inv_n)
    nc.vector.tensor_scalar_mul(out=var[:pp], in0=s_sq[:pp], scalar1=inv_n)
    nc.vector.tensor_tensor(out=t[:pp], in0=mean[:pp], in1=mean[:pp], op=ALU.mult)
    nc.vector.tensor_sub(out=var[:pp], in0=var[:pp], in1=t[:pp])
    nc.vector.tensor_scalar_add(out=var[:pp], in0=var[:pp], scalar1=EPS)
    nc.scalar.activation(out=sd[:pp], in_=var[:pp], func=ACTF.Sqrt, bias=0.0)
    nc.vector.reciprocal(out=r[:pp], in_=sd[:pp])
    nc.vector.tensor_tensor(out=s1[:pp], in0=g_ap, in1=r[:pp], op=ALU.mult)
    nc.vector.tensor_tensor(out=t[:pp], in0=mean[:pp], in1=s1[:pp], op=ALU.mult)
    nc.vector.tensor_sub(out=s2[:pp], in0=be_ap, in1=t[:pp])
    nc.scalar.activation(out=out_t, in_=src_f32, func=ACTF.Relu,
                         bias=s2[:pp], scale=s1[:pp])


def _build(loop_reps=1, skip_tr=False):
    nc = bass.Bass()
    dp = nc.declare_dram_parameter
    xT = dp("xT", [ND, B], F32R, isOutput=False)
    wb0t = dp("wb0t", [ND, 512], F32R, isOutput=False)
    wb1t = dp("wb1t", [512, 256], BF16, isOutput=False)
    wb2t = dp("wb2t", [256, 16], BF16, isOutput=False)
    wt1t = dp("wt1t", [512, 256], BF16, isOutput=False)
    wt2t = dp("wt2t", [256, 1], BF16, isOutput=False)
    gb0 = dp("gb0", [512, 1], F32, isOutput=False)
    beb0 = dp("beb0", [512, 1], F32, isOutput=False)
    gb1 = dp("gb1", [256, 1], F32, isOutput=False)
    beb1 = dp("beb1", [256, 1], F32, isOutput=False)
    gb2 = dp("gb2", [16, 1], F32, isOutput=False)
    beb2 = dp("beb2", [16, 1], F32, isOutput=False)
    gt0 = dp("gt0", [512, 1], F32, isOutput=False)
    bet0 = dp("bet0", [512, 1], F32, isOutput=False)
    gt1 = dp("gt1", [256, 1], F32, isOutput=False)
    bet1 = dp("bet1", [256, 1], F32, isOutput=False)
    bt2p = dp("bt2", [1, 1], F32, isOutput=False)
    estack = dp("estack", [ESTACK, D], BF16, isOutput=False)
    gidx = dp("gidx", [P, NBT * SLOTS], I32, isOutput=False)
    sidx = dp("sidx", [P, 4], I32, isOutput=False)
    fidx = dp("fidx", [P, 4], I32, isOutput=False)
    w0t = dp("w0t", [SHARD_K, 512], BF16, isOutput=False)
    w0d = dp("w0d", [16, 512], BF16, isOutput=False)
    identp = dp("ident", [P, P], F32R, isOutput=False)
    out_p = dp("out", [1, B], F32, isOutput=True)

    ag_in = nc.dram_tensor("ag_in", [64, B], BF16)
    ag_out = nc.dram_tensor("ag_out", [512, B], BF16, addr_space="Shared")
    fdt = nc.dram_tensor("fdt", [FDT_ROWS, B], BF16)
    ar_in = [nc.dram_tensor(f"ar_in{h}", [512, 512], F32) for h in range(2)]
    ar_out = [nc.dram_tensor(f"ar_out{h}", [512, 512], F32, addr_space="Shared")
              for h in range(2)]

    rg = [list(range(NCORE))]

    with ExitStack() as ctx:
        tc = ctx.enter_context(tile.TileContext(nc))
        const = ctx.enter_context(tc.tile_pool(name="const", bufs=1))
        sb = ctx.enter_context(tc.tile_pool(name="sb", bufs=2))
        pers = ctx.enter_context(tc.tile_pool(name="pers", bufs=1))

        dma = nc.sync.dma_start

        identr = const.tile([P, P], F32R, tag="identr", name="identr")
        ident32 = const.tile([P, P], F32, tag="ident32", name="ident32")
        identb = const.tile([P, P], BF16, tag="identb", name="identb")
        dma(out=identr[:], in_=identp[:])
        nc.vector.tensor_copy(out=ident32[:], in_=identr[:].bitcast(F32))
        nc.vector.tensor_copy(out=identb[:], in_=identr[:].bitcast(F32))

        # W0 shard, bf16, fully SBUF-resident: 12 k-group tiles of [128, 8*512]
        w0g = []
        for g in range(NKGP):
            w0g_g = const.tile([P, 8 * 512], BF16, tag=f"w0g{g}", name=f"w0g{g}")
            dma(out=w0g_g[:].rearrange("p (kk o) -> p kk o", kk=8),
                in_=w0t[g * 1024:(g + 1) * 1024, :].rearrange("(kk p) o -> p kk o", p=P))
            w0g.append(w0g_g)
        w0d_sb = const.tile([16, 512], BF16, tag="w0d", name="w0d")
        dma(out=w0d_sb[:], in_=w0d[:])

        # ---------- bottom MLP (feature-major, full batch, fp32) ----------
        xT_sb = const.tile([ND, B], F32R, tag="xT", name="xT")
        dma(out=xT_sb[:], in_=xT[:])
        wb0_sb = const.tile([ND, 512], F32R, tag="wb0", name="wb0")
        dma(out=wb0_sb[:], in_=wb0t[:])
        # wb1t [512,256] as [128, 4*256] (k-chunk major)
        wb1_sb = const.tile([P, 4 * 256], BF16, tag="wb1", name="wb1")
        dma(out=wb1_sb[:].rearrange("p (k m) -> p k m", k=4), in_=wb1t[:].rearrange("(k p) m -> p k m", p=P))
        wb2_sb = const.tile([P, 2 * 16], BF16, tag="wb2", name="wb2")
        dma(out=wb2_sb[:].rearrange("p (k m) -> p k m", k=2), in_=wb2t[:].rearrange("(k p) m -> p k m", p=P))
        wt1_sb = const.tile([P, 4 * 256], BF16, tag="wt1", name="wt1")
        dma(out=wt1_sb[:].rearrange("p (k m) -> p k m", k=4), in_=wt1t[:].rearrange("(k p) m -> p k m", p=P))
        wt2_sb = const.tile([P, 2], BF16, tag="wt2", name="wt2")
        dma(out=wt2_sb[:].rearrange("p (k m) -> p k m", k=2), in_=wt2t[:].rearrange("(k p) m -> p k m", p=P))

        bn_sc = {}
        for nm, prm, rows in (("gb0", gb0, 512), ("beb0", beb0, 512),
                              ("gb1", gb1, 256), ("beb1", beb1, 256),
                              ("gb2", gb2, 16), ("beb2", beb2, 16),
                              ("gt0", gt0, 512), ("bet0", bet0, 512),
                              ("gt1", gt1, 256), ("bet1", bet1, 256)):
            nchunk = (rows + P - 1) // P
            t_ = const.tile([P, nchunk], F32, tag=f"sc_{nm}", name=f"sc_{nm}")
            rp = min(rows, P)
            dma(out=t_[:rp, :].rearrange("p (k one) -> p k one", one=1),
                in_=prm[:].rearrange("(k p) one -> p k one", p=rp))
            bn_sc[nm] = t_

        with tc.tile_pool(name="ps_a", bufs=2, space="PSUM") as ps:
            # L0: h0T [512, 1024]
            h0r = []
            for oc in range(4):
                h0ps = ps.tile([P, 512], F32, tag="y0ps", name="h0ps", bufs=3)
                h0ps2 = ps.tile([P, 512], F32, tag="y0ps", name="h0ps2", bufs=3)
                nc.tensor.matmul(h0ps[:], wb0_sb[:, oc * P:(oc + 1) * P], xT_sb[:, 0:512],
                                 start=True, stop=True)
                nc.tensor.matmul(h0ps2[:], wb0_sb[:, oc * P:(oc + 1) * P], xT_sb[:, 512:1024],
                                 start=True, stop=True)
                h0 = sb.tile([P, B], F32, tag="h0", name="h0")
                nc.vector.tensor_copy(out=h0[:, 0:512], in_=h0ps[:])
                nc.vector.tensor_copy(out=h0[:, 512:1024], in_=h0ps2[:])
                h0r_oc = pers.tile([P, B], BF16, tag=f"h0r{oc}", name=f"h0r{oc}")
                _bn_relu(nc, sb, h0[:], B, bn_sc["gb0"][:, oc:oc + 1], bn_sc["beb0"][:, oc:oc + 1], h0r_oc[:])
                h0r.append(h0r_oc)

            # L1: h1T [256, 1024] = wb1 @ h0r
            h1r = []
            for mo in range(2):
                h1 = sb.tile([P, B], F32, tag="h1", name="h1", bufs=1)
                for bh in range(2):
                    h1ps = ps.tile([P, 512], F32, tag="y0ps", name="h1ps", bufs=3)
                    for k in range(4):
                        nc.tensor.matmul(
                            h1ps[:],
                            wb1_sb[:, k * 256 + mo * P: k * 256 + (mo + 1) * P],
                            h0r[k][:, bh * 512:(bh + 1) * 512],
                            start=(k == 0), stop=(k == 3))
                    nc.vector.tensor_copy(out=h1[:, bh * 512:(bh + 1) * 512], in_=h1ps[:])
                h1r_m = pers.tile([P, B], BF16, tag=f"h1r{mo}", name=f"h1r{mo}")
                _bn_relu(nc, sb, h1[:], B, bn_sc["gb1"][:, mo:mo + 1], bn_sc["beb1"][:, mo:mo + 1], h1r_m[:])
                h1r.append(h1r_m)

            # L2: dT [16, 1024]
            d_f32 = sb.tile([16, B], F32, tag="d_f32", name="d_f32", bufs=1)
            for bh in range(2):
                dps = ps.tile([16, 512], F32, tag="y0ps", name="dps", bufs=3)
                for k in range(2):
                    nc.tensor.matmul(dps[:], wb2_sb[:, k * 16:(k + 1) * 16],
                                     h1r[k][:, bh * 512:(bh + 1) * 512],
                                     start=(k == 0), stop=(k == 1))
                nc.vector.tensor_copy(out=d_f32[:, bh * 512:(bh + 1) * 512], in_=dps[:])
            dTb = pers.tile([16, B], BF16, tag="dTb", name="dTb")
            _bn_relu(nc, sb, d_f32[:], B, bn_sc["gb2"][:16, 0:1], bn_sc["beb2"][:16, 0:1], dTb[:])

            # ---------- embedding gather + feature-major AllGather (bf16) ----------
            gidx_sb = const.tile([P, NBT * SLOTS], I32, tag="gidx", name="gidx")
            dma(out=gidx_sb[:], in_=gidx[:])
            slabT = pers.tile([64, B], BF16, tag="slabT", name="slabT")
            for bt in range(NBT):
                slab = sb.tile([P, 64], BF16, tag="slab", name="slab")
                for s in range(SLOTS):
                    nc.gpsimd.indirect_dma_start(
                        out=slab[:, s * D:(s + 1) * D],
                        out_offset=None,
                        in_=estack[:],
                        in_offset=bass.IndirectOffsetOnAxis(
                            ap=gidx_sb[:, bt * SLOTS + s: bt * SLOTS + s + 1], axis=0),
                    )
                trp = ps.tile([64, P], BF16, tag="trps", name="slabtr", bufs=3)
                nc.tensor.transpose(out=trp[:], in_=slab[:], identity=identb[:])
                nc.vector.tensor_copy(out=slabT[:, bt * P:(bt + 1) * P], in_=trp[:])
            dma(out=ag_in[:], in_=slabT[:])
            nc.gpsimd.collective_compute(
                "AllGather", ALU.bypass, ins=[ag_in[:]], outs=[ag_out[:]], replica_groups=rg)

            # ---------- featDupT staging (feature-major f_cyc, bf16) ----------
            dma(out=fdt[0:16, :], in_=dTb[:])
            dma(out=fdt[432:448, :], in_=dTb[:])
            for c in range(NCORE):
                nreal = TCNT[c] * D
                dma(out=fdt[16 + T0[c] * D: 16 + T0[c] * D + nreal, :],
                    in_=ag_out[64 * c: 64 * c + nreal, :])
            # dup rows 448..648 = emb features 0..199 (+649 clamp row)
            dma(out=fdt[448:512, :], in_=ag_out[0:64, :])       # tables 0..3
            dma(out=fdt[512:576, :], in_=ag_out[64:128, :])     # tables 4..7
            dma(out=fdt[576:624, :], in_=ag_out[128:176, :])    # tables 8..10
            dma(out=fdt[624:648, :], in_=ag_out[192:216, :])    # tables 11..12(:8)
            dma(out=fdt[648:649, :], in_=ag_out[216:217, :])    # core7 clamp row

            # ---------- feat / S (batch-major, bf16) via row-gather + PE transpose ----------
            sidx_sb = const.tile([P, 4], I32, tag="sidx", name="sidx")
            fidx_sb = const.tile([P, 4], I32, tag="fidx", name="fidx")
            dma(out=sidx_sb[:], in_=sidx[:])
            dma(out=fidx_sb[:], in_=fidx[:])

            featbt = [pers.tile([P, 512], BF16, tag=f"feat{bt}", name=f"feat{bt}") for bt in range(NBT)]
            sbt = [pers.tile([P, 512], BF16, tag=f"sc{bt}", name=f"sc{bt}") for bt in range(NBT)]
            for (idx_sb, dest) in ((fidx_sb, featbt), (sidx_sb, sbt)):
                for q in range(4):
                    gt = sb.tile([P, B], BF16, tag="rowg", name="rowg")
                    nc.gpsimd.indirect_dma_start(
                        out=gt[:], out_offset=None, in_=fdt[:],
                        in_offset=bass.IndirectOffsetOnAxis(ap=idx_sb[:, q:q + 1], axis=0))
                    for bt in range(NBT):
                        trp = ps.tile([P, P], BF16, tag="trps", name="rowtr", bufs=3)
                        nc.tensor.transpose(out=trp[:], in_=gt[:, bt * P:(bt + 1) * P],
                                            identity=identb[:])
                        nc.vector.tensor_copy(out=dest[bt][:, q * P:(q + 1) * P], in_=trp[:])

        # ---------- main loop: bt outer, y0 accumulated in one PSUM bank ----------
        # segment lists per k-group (free-axis offsets, structural / SPMD-safe)
        seglists = []
        for g in range(NKGP):
            segs = []
            p0 = 1024 * g
            while p0 < 1024 * (g + 1):
                m_ = p0 // F
                i0 = p0 % F
                L = min(F - i0, 1024 * (g + 1) - p0)
                segs.append((p0 - 1024 * g, m_, i0, L))
                p0 += L
            seglists.append(segs)

        y0sb = [pers.tile([P, 512], F32, tag=f"y0sb{bt}", name=f"y0sb{bt}") for bt in range(4)]
        with tc.tile_pool(name="ps_m", bufs=2, space="PSUM") as psm:
            for bt in range(NBT):
                y0ps = psm.tile([P, 512], F32, tag="y0acc", name=f"y0acc{bt}", bufs=2)
                # dense part of top W0 (w0d is zero on cores 1..7) opens the chain
                nc.tensor.matmul(y0ps[:], dTb[:, bt * P:(bt + 1) * P], w0d_sb[:],
                                 start=True, stop=False, skip_group_check=True)
                for g0 in range(NKGP * loop_reps):
                    g = g0 % NKGP
                    ib = sb.tile([P, 1024], BF16, tag="ib", name="ib", bufs=3)
                    for (off, m_, i0, L) in seglists[g]:
                        nc.vector.tensor_tensor(
                            out=ib[:, off:off + L],
                            in0=featbt[bt][:, i0:i0 + L],
                            in1=sbt[bt][:, i0 + m_: i0 + m_ + L],
                            op=ALU.mult)
                    rt = sb.tile([P, 1024], BF16, tag="rt", name="rt", bufs=3)
                    for half in range(2):
                        trp = psm.tile([P, 512], BF16, tag="trps", name="trps", bufs=4)
                        for kk in range(4):
                            nc.tensor.transpose(
                                out=trp[:, kk * P:(kk + 1) * P],
                                in_=ib[:, half * 512 + kk * P: half * 512 + (kk + 1) * P],
                                identity=identb[:])
                        nc.vector.tensor_copy(out=rt[:, half * 512:(half + 1) * 512], in_=trp[:])
                    last = g0 == NKGP * loop_reps - 1
                    for kk in range(8):
                        nc.tensor.matmul(
                            y0ps[:], rt[:, kk * P:(kk + 1) * P],
                            w0g[g][:, kk * 512:(kk + 1) * 512],
                            start=False, stop=(last and kk == 7), skip_group_check=True)
                nc.vector.tensor_copy(out=y0sb[bt % 4][:], in_=y0ps[:])

                # after each batch half completes: transpose to feature-major + AllReduce
                if bt == 3 or bt == 7:
                    h = bt // 4
                    for oc in range(4):
                        yts = sb.tile([P, 512], F32, tag="yts", name="yts")
                        ytp = psm.tile([P, 512], F32, tag="artr", name="ytp", bufs=2)
                        for bt4 in range(4):
                            nc.tensor.transpose(out=ytp[:, bt4 * P:(bt4 + 1) * P],
                                                in_=y0sb[bt4][:, oc * P:(oc + 1) * P],
                                                identity=ident32[:])
                        nc.vector.tensor_copy(out=yts[:], in_=ytp[:])
                        dma(out=ar_in[h][oc * P:(oc + 1) * P, :], in_=yts[:])
                    nc.gpsimd.collective_compute(
                        "AllReduce", ALU.add, ins=[ar_in[h][:]], outs=[ar_out[h][:]],
                        replica_groups=rg)

        # ---------- top MLP (fp32) ----------
        with tc.tile_pool(name="ps_b", bufs=2, space="PSUM") as ps:
            y1r = []
            for oc in range(4):
                y0T = sb.tile([P, B], F32, tag="h0", name="y0T")
                for h in range(2):
                    dma(out=y0T[:, h * 512:(h + 1) * 512],
                        in_=ar_out[h][oc * P:(oc + 1) * P, :])
                y1r_oc = pers.tile([P, B], BF16, tag=f"h0r{oc}", name=f"y1r{oc}")
                _bn_relu(nc, sb, y0T[:], B, bn_sc["gt0"][:, oc:oc + 1], bn_sc["bet0"][:, oc:oc + 1], y1r_oc[:])
                y1r.append(y1r_oc)

            y2r = []
            for mo in range(2):
                y2 = sb.tile([P, B], F32, tag="h0", name="y2")
                for bh in range(2):
                    y2ps = ps.tile([P, 512], F32, tag="y0ps", name="y2ps", bufs=3)
                    for k in range(4):
                        nc.tensor.matmul(
                            y2ps[:],
                            wt1_sb[:, k * 256 + mo * P: k * 256 + (mo + 1) * P],
                            y1r[k][:, bh * 512:(bh + 1) * 512],
                            start=(k == 0), stop=(k == 3))
                    nc.vector.tensor_copy(out=y2[:, bh * 512:(bh + 1) * 512], in_=y2ps[:])
                y2r_m = pers.tile([P, B], BF16, tag=f"h1r{mo}", name=f"y2r{mo}")
                _bn_relu(nc, sb, y2[:], B, bn_sc["gt1"][:, mo:mo + 1], bn_sc["bet1"][:, mo:mo + 1], y2r_m[:])
                y2r.append(y2r_m)

            bt2_sb = const.tile([1, 1], F32, tag="bt2", name="bt2")
            dma(out=bt2_sb[:], in_=bt2p[:])
            o_sb = sb.tile([1, B], F32, tag="o_sb", name="o_sb")
            for bh in range(2):
                opsb = ps.tile([1, 512], F32, tag="y0ps", name="opsb", bufs=3)
                for k in range(2):
                    nc.tensor.matmul(
                        opsb[:],
                        wt2_sb[:, k:k + 1],
                        y2r[k][:, bh * 512:(bh + 1) * 512],
                        start=(k == 0), stop=(k == 1))
                nc.vector.tensor_scalar_add(out=o_sb[:, bh * 512:(bh + 1) * 512], in0=opsb[:], scalar1=bt2_sb[0:1, 0:1])
            dma(out=out_p[:], in_=o_sb[:])

    _split_waits(nc)
    return nc


# =====================================================================
# runner (PJRT via axon, cached jit)
# =====================================================================
def _get_runner():
    if "runner" in _CACHE:
        return _CACHE["runner"]
    import jax
    from jax.sharding import Mesh, PartitionSpec
    from jax.experimental.shard_map import shard_map
    from concourse import bass2jax
    from concourse.bass2jax import _bass_exec_p, partition_id_tensor

    bass2jax.install_neuronx_cc_hook()
    nc = _build()

    partition_name = nc.partition_id_tensor.name if nc.partition_id_tensor else None
    in_names, out_names, out_avals, zero_outs = [], [], [], []
    for alloc in nc.m.functions[0].allocations:
        if not isinstance(alloc, mybir.MemoryLocationSet):
            continue
        name = alloc.memorylocations[0].name
        if alloc.kind == "ExternalInput":
            if name != partition_name:
                in_names.append(name)
        elif alloc.kind == "ExternalOutput":
            out_names.append(name)
            shape = tuple(alloc.tensor_shape)
            dtype = mybir.dt.np(alloc.dtype)
            out_avals.append(jax.core.ShapedArray(shape, dtype))
            zero_outs.append(np.zeros(shape, dtype))
    n_params = len(in_names)
    n_outs = len(out_avals)
    in_names_all = in_names + out_names
    if partition_name is not None:
        in_names_all.append(partition_name)

    def _body(*args):
        operands = list(args)
        if partition_name is not None:
            operands.append(partition_id_tensor())
        outs = _bass_exec_p.bind(
            *operands,
            out_avals=tuple(out_avals),
            in_names=tuple(in_names_all),
            out_names=tuple(out_names),
            lowering_input_output_aliases=(),
            sim_require_finite=True,
            sim_require_nnan=True,
            nc=nc,
        )
        return tuple(outs)

    devices = jax.devices()[:NCORE]
    mesh = Mesh(np.asarray(devices), ("core",))
    in_specs = (PartitionSpec("core"),) * (n_params + n_outs)
    out_specs = (PartitionSpec("core"),) * n_outs
    sharded = jax.jit(
        shard_map(_body, mesh=mesh, in_specs=in_specs, out_specs=out_specs,
                  check_rep=False),
        keep_unused=True,
    )
    runner = {
        "sharded": sharded, "in_names": in_names, "out_names": out_names,
        "zero_outs": zero_outs, "out_avals": out_avals, "body": _body,
    }
    _CACHE["nc"] = nc
    _CACHE["runner"] = runner
    return runner


def run_cores(per_core_maps):
    """per_core_maps: list of 8 dicts name->np.ndarray. Returns list of out dicts."""
    r = _get_runner()
    concat_in = [
        np.concatenate([per_core_maps[c][n] for c in range(NCORE)], axis=0)
        for n in r["in_names"]
    ]
    concat_zeros = [np.zeros((NCORE * z.shape[0], *z.shape[1:]), z.dtype)
                    for z in r["zero_outs"]]
    _CACHE["last_concat_in"] = concat_in
    out_arrs = r["sharded"](*concat_in, *concat_zeros)
    return [
        {n: np.asarray(out_arrs[i]).reshape(NCORE, *r["out_avals"][i].shape)[c]
         for i, n in enumerate(r["out_names"])}
        for c in range(NCORE)
    ]


def kernel(**inputs) -> np.ndarray:
    per_core = [_prep_core(c, inputs) for c in range(NCORE)]
    res = run_cores(per_core)
    return np.asarray(res[0]["out"], np.float32).reshape(B, 1)


def bench(n=5):
    """Re-run the last inputs with device-resident buffers; return min seconds."""
    import time
    import jax
    from jax.sharding import Mesh, PartitionSpec, NamedSharding
    r = _get_runner()
    concat_in = _CACHE["last_concat_in"]
    devices = jax.devices()[:NCORE]
    mesh = Mesh(np.asarray(devices), ("core",))
    sh = NamedSharding(mesh, PartitionSpec("core"))
    dev_in = [jax.device_put(a, sh) for a in concat_in]
    concat_zeros = [np.zeros((NCORE * z.shape[0], *z.shape[1:]), z.dtype)
                    for z in r["zero_outs"]]
    times = []
    for _ in range(n):
        z = [jax.device_put(np.copy(zz), sh) for zz in concat_zeros]
        jax.block_until_ready(dev_in)
        jax.block_until_ready(z)
        t0 = time.perf_counter()
        out = r["sharded"](*dev_in, *z)
        jax.block_until_ready(out)
        times.append(time.perf_counter() - t0)
    return min(times), times
